# revision 17
# baseline (speedup 1.0000x reference)
"""Distance-aware comb-pilot interpolator for Trainium2 (8 NeuronCores).

Math: out[b, i, c] = (w_l[i] * H[b, j0(i), c] + w_r[i] * H[b, j1(i), c]) / w[i]
with pilots on the comb loc[k] = 8k (k = 0..511), Nfft = 4096.  For
i = 8k + r the normalized weights depend only on r, so each 128-subcarrier
block of the output is the SAME banded 17x128 matrix W applied to 17
consecutive pilots: out[128m + 8kk + r] = alpha[r] H[16m+kk] + gamma[r]
H[16m+kk+1].  The last block folds the reference's extrapolated virtual
pilot hN = (15/8)H[511] - (7/8)H[510] into per-r coefficients on
H[510]/H[511] (a second 16x128 stationary matrix).

Device kernel (per core, batch-sharded 512 rows): one TensorE matmul per
(channel, m) chunk computes 128 subcarriers x 512 batch into PSUM; DVE and
ACT alternate evacuating psum to fp16 SBUF; chunked HWDGE stores stream the
8.4 MB fp16 output (half the f32 bytes -- the fp16 round-trip costs ~1e-3
relative error against a 2e-2 gate).

Schedule notes (v8, distilled from v1-v7 traces):
- ~17 us of runtime is fixed wrapper cost (engine barriers, TENSOR_LOADs,
  a mandatory ~254-instruction semaphore-reset epilogue): trivial kernel
  = 20.0 us.  Only the work phase is addressable.
- The store stream is SBUF-AXI-port-bound at ~427 GB/s (16 ports x
  27 GB/s); 8.4 MB takes ~19.6 us and dominates.  Loads into the
  17-partition band tiles touch only ~5 ports per band and pay a per-DMA
  sem-receipt barrier + non-pipelined HBM-read stalls, so the two bands
  load CONCURRENTLY: band 64 on sync (fastest ring), band 0 on gpsimd.
  The scalar ring is the slowest bulk loader (v3/v6: ~66-90 GB/s; a v6
  attempt to put band-0 there stalled the PE 5.3 us).
- Weights live in a DEDICATED tile: embedding them in the band tiles
  (v6/v7) made the Tile scheduler coarsen every LDWEIGHTS dependency
  onto the band's whole load stream (PE stalled until the last strip
  landed ~18 us).  But the weights DMA must not be one 81-row x 512 B
  monolith either -- its read-stalled descriptors gated the first matmul
  at ~11.4-12.2 us in v1-v5.  v8 loads the two 17-row halves as two
  small DMAs on the otherwise-idle scalar ring: w64 lands ~9.3 us.
- gpsimd (SWDGE) descriptor-ring traffic makes SDMA engine 15 lag the
  store tail; keeping all SWDGE work finished by ~16 us (fat pieces,
  strips split fine) keeps that tail ~1 us.
"""

import sys

import numpy as np

for _p in ("/opt/trn_rl_repo", "/root/.axon_site/_ro/trn_rl_repo"):
    if _p not in sys.path:
        sys.path.append(_p)

import concourse.bass as bass
import concourse.tile as tile
from concourse import bacc, mybir
from concourse.bass_utils import run_bass_kernel_spmd

N_CORES = 8
B, NP, NFFT, SPACING = 4096, 512, 4096, 8
B_LOC = B // N_CORES  # batch rows per core
P = 128  # SBUF partitions
NCHUNK = 64  # (ch, m) chunks: ch = q // 32 (re/im), m = q % 32 (128-subcarrier block)
MG = 16  # chunk slots per band
ROW = 2 * MG * B_LOC  # ls row length in fp16 elements (both channels)

# chunks per output store; small leading groups start the store stream
# early, 8-chunk middle groups give 8 KB-run descriptors (line rate),
# small trailing groups keep the post-compute drain short.
STORE_GROUPS = [2, 2, 4, 8, 8, 8, 8, 8, 8, 4, 2, 2]
assert sum(STORE_GROUPS) == NCHUNK

_PROGRAM = None


def _band_slot(m: int) -> tuple[int, int]:
    """SBUF band (partition base) and column slot of chunk m.

    Odd chunks (and m=31) live at partitions 0..16, even chunks at
    64..80 -- consecutive m alternate PE row-groups so back-to-back
    matmuls overlap in the array."""
    if m == 31:
        return 0, 15
    return (0, m // 2) if m % 2 else (64, m // 2)


def _build_program():
    """One Bass program, identical on all cores (pure data parallel)."""
    nc = bacc.Bacc("TRN2", target_bir_lowering=False, debug=False)
    f16 = mybir.dt.float16
    f32 = mybir.dt.float32
    # ls[17*band2 + j, ch*8192 + m2*512 + b]: band-major pre-gathered
    # layout (band2 0 = odd chunks -> partitions 0..16, band2 1 = even ->
    # 64..80).  Row = one partition's contiguous 32 KB payload.
    ls = nc.dram_tensor("ls", [34, ROW], f16, kind="ExternalInput").ap()
    # wm[81, 256]: cols 0:128 = W17 at rows 0..16 AND 64..80; cols
    # 128:256 rows 0..16 = the hN-folded last-chunk band.
    wm = nc.dram_tensor("wm", [81, 2 * P], f16, kind="ExternalInput").ap()
    # out[p, q*512 + b]: subcarrier-position p = 8*kk + r of chunk q = ch*32 + m.
    out = nc.dram_tensor("out", [P, NCHUNK * B_LOC], f16, kind="ExternalOutput").ap()

    with tile.TileContext(nc) as tc:
        with (
            tc.tile_pool(name="wpool", bufs=1) as wpool,
            tc.tile_pool(name="lpool", bufs=1) as lpool,
            tc.psum_pool(name="ppool", bufs=4) as ppool,
            tc.tile_pool(name="opool", bufs=6) as opool,
        ):
            # Dedicated weight tile -> LDWEIGHTS depends only on these two
            # small DMAs, never on the band data stream.
            wb = wpool.tile([81, 2 * P], f16, name="wb", tag="wb")
            nc.scalar.dma_start(wb[64:81, :], wm[64:81, :])
            nc.scalar.dma_start(wb[0:17, :], wm[0:17, :])

            # One tile per band holding both channels.  Mains are [16,
            # cols]; the 17th row (j=16) goes as [1, cols] strips (its
            # partition, 16 / 80, sits on its own SBUF AXI port, so
            # strips never contend with mains).  17-partition DMA
            # destinations would break the descriptor spray, hence 16+1.
            lts = {
                band: lpool.tile(
                    [81 if band else 17, ROW], f16,
                    name=f"ls{band}", tag=f"ls{band}",
                )
                for band in (64, 0)
            }

            def main(ring, band, c0, c1):
                lt = lts[band]
                r = 17 if band else 0
                ring.dma_start(lt[band : band + 16, c0:c1], ls[r : r + 16, c0:c1])

            def strip(ring, band, c0, c1):
                lt = lts[band]
                r = (17 if band else 0) + 16
                ring.dma_start(lt[band + 16 : band + 17, c0:c1], ls[r : r + 1, c0:c1])

            def col(ch, s):  # ls column of (channel, slot) batch block start
                return (ch * MG + s) * B_LOC

            # Pieces sized to the PE's slot-consumption deadlines: A =
            # ch0 slots 0-3 (gates the first matmuls), B1/B2 = ch0 rest.
            # Band 64 ch0 rides sync, band 0 ch0 rides gpsimd: the bands'
            # disjoint SBUF-port groups (1,3,5,7 vs 0,2,4,6) drain
            # concurrently, and both fast rings finish by ~12.5-13.5 us
            # so the store stream owns q1 early.  ch1 (both bands, not
            # consumed until ~20 us) rides the slow-but-idle scalar ring
            # after the weights (~70 GB/s moves 0.52 MB by ~16 us, well
            # inside the deadline).  Strips mirror the mains; their
            # ports (8/9) are separate, so they never contend.
            CUTS = [col(0, 0), col(0, 4), col(0, 10), col(1, 0)]
            for band, ring in ((64, nc.sync), (0, nc.gpsimd)):
                for c0, c1 in zip(CUTS[:-1], CUTS[1:]):
                    main(ring, band, c0, c1)
                    strip(ring, band, c0, c1)
            for band in (64, 0):
                main(nc.scalar, band, col(1, 0), ROW)
                strip(nc.scalar, band, col(1, 0), ROW)

            q = 0
            pair = 0
            for gn in STORE_GROUPS:
                o = opool.tile([P, gn * B_LOC], f16)
                q0 = q
                for j in range(0, gn, 2):
                    # one 2-bank psum tile per chunk pair: the paired evac
                    # (FD 1024) amortizes the per-op fixed cost and halves
                    # evac semaphore traffic.
                    ps = ppool.tile([P, 2 * B_LOC], f32)
                    for h in range(2):
                        ch, m = q // 32, q % 32
                        band, m2 = _band_slot(m)
                        if m == 31:
                            # last chunk: pilots 496..511, hN-folded band
                            lhsT, nrows = wb[0:16, P : 2 * P], 16
                        else:
                            lhsT, nrows = wb[band : band + 17, 0:P], 17
                        lt = lts[band]
                        c0 = col(ch, m2)
                        nc.tensor.matmul(
                            ps[:, h * B_LOC : (h + 1) * B_LOC],
                            lhsT,
                            lt[band : band + nrows, c0 : c0 + B_LOC],
                            start=True,
                            stop=True,
                        )
                        q += 1
                    # psum -> fp16 SBUF, alternating engines so consecutive
                    # pairs drain in parallel (DVE ~1.2us, ACT ~1.1us).
                    osl = o[:, j * B_LOC : (j + 2) * B_LOC]
                    if pair % 2 == 0:
                        nc.vector.tensor_copy(osl, ps[:])
                    else:
                        nc.scalar.copy(osl, ps[:])
                    pair += 1
                nc.sync.dma_start(out[:, q0 * B_LOC : q * B_LOC], o[:])
    nc.compile()
    return nc


def _w_mats(decay_param) -> np.ndarray:
    """[81, 256] fp16: cols 0:128 = W17[j, 8kk+r] = alpha[r] (j=kk) /
    gamma[r] (j=kk+1) at rows 0..16 and duplicated at rows 64..80; cols
    128:256 rows 0..15 = the last-chunk band (kk=15 columns use the
    hN-folded coefficients on pilots 510/511)."""
    x = float(np.asarray(decay_param).reshape(-1)[0])
    d = float(np.logaddexp(0.0, x))  # softplus
    r = np.arange(SPACING, dtype=np.float64)
    eps = 1e-12
    wl = np.exp(-d * r)
    wr = np.exp(-d * (float(SPACING) - r))
    w = wl + wr + eps
    alpha, gamma = wl / w, wr / w
    # last 8 subcarriers: i = 4088 + r, x0 = 4088, x1 = 4095 (gap of 7);
    # y1 = hN = (15/8) H[511] - (7/8) H[510]
    wl2 = np.exp(-d * r)
    wr2 = np.exp(-d * (7.0 - r))
    w2 = wl2 + wr2 + eps
    c511 = (wl2 + 1.875 * wr2) / w2
    c510 = -0.875 * wr2 / w2
    W17 = np.zeros((17, P), np.float64)
    Wlast = np.zeros((16, P), np.float64)
    cols = np.arange(SPACING)
    for kk in range(16):
        W17[kk, 8 * kk + cols] = alpha
        W17[kk + 1, 8 * kk + cols] = gamma
    for kk in range(15):
        Wlast[kk, 8 * kk + cols] = alpha
        Wlast[kk + 1, 8 * kk + cols] = gamma
    Wlast[14, 120:128] = c510
    Wlast[15, 120:128] = c511
    W = np.zeros((81, 2 * P), np.float16)
    W[0:17, 0:P] = W17.astype(np.float16)
    W[64:81, 0:P] = W17.astype(np.float16)
    W[0:16, P : 2 * P] = Wlast.astype(np.float16)
    return W


def _gather_ls4(shard: np.ndarray) -> np.ndarray:
    """[34, 16384] fp16 band-major: row 17*band2 + j holds partition
    (band2 ? 0 : 64) + j's payload -- cols ch*8192 + slot*512 + b, one
    contiguous 32 KB DRAM run per partition (few fat load DMAs)."""
    lsT = shard.transpose(2, 1, 0).astype(np.float16).reshape(2 * NP, B_LOC)
    j = np.arange(17)[:, None]  # [17, 1]
    m_odd = np.array([2 * m2 + 1 for m2 in range(15)] + [31])  # band2=0 slots
    m_even = np.arange(0, 32, 2)  # band2=1 slots
    rows_odd = np.minimum(16 * m_odd[None, :] + j, NP - 1)  # clip m31 j=16
    rows_even = 16 * m_even[None, :] + j
    out = np.empty((34, ROW), np.float16)
    for ch in range(2):
        base = 512 * ch
        cols = slice(ch * MG * B_LOC, (ch + 1) * MG * B_LOC)
        out[0:17, cols] = lsT[base + rows_odd].reshape(17, -1)
        out[17:34, cols] = lsT[base + rows_even].reshape(17, -1)
    return out


def kernel(LS_ri, pilot_pos=None, decay_param=None, Nfft=None, **_unused):
    global _PROGRAM
    LS_ri = np.asarray(LS_ri, dtype=np.float32)
    Wm = _w_mats(decay_param)

    if _PROGRAM is None:
        _PROGRAM = _build_program()
    nc = _PROGRAM

    in_maps = []
    for c in range(N_CORES):
        shard = LS_ri[c * B_LOC : (c + 1) * B_LOC]  # [512, 512, 2]
        in_maps.append({"ls": _gather_ls4(shard), "wm": Wm})

    res = run_bass_kernel_spmd(nc, in_maps, list(range(N_CORES))).results
    outs = []
    for c in range(N_CORES):
        a = np.asarray(res[c]["out"]).reshape(16, 8, 2, 32, B_LOC)  # kk r ch m b
        a = a.transpose(4, 3, 0, 1, 2).reshape(B_LOC, NFFT, 2)
        outs.append(a.astype(np.float32))
    return np.concatenate(outs, axis=0)


# revision 21
# speedup vs baseline: 1.1351x; 1.1351x over previous
"""Distance-aware comb-pilot interpolator for Trainium2 (8 NeuronCores).

Math: out[b, i, c] = (w_l[i] * H[b, j0(i), c] + w_r[i] * H[b, j1(i), c]) / w[i]
with pilots on the comb loc[k] = 8k (k = 0..511), Nfft = 4096.  For
i = 8k + r the normalized weights depend only on r, so each 128-subcarrier
block of the output is the SAME banded 17x128 matrix W applied to 17
consecutive pilots: out[128m + 8kk + r] = alpha[r] H[16m+kk] + gamma[r]
H[16m+kk+1].  The last block folds the reference's extrapolated virtual
pilot hN = (15/8)H[511] - (7/8)H[510] into per-r coefficients on
H[510]/H[511] (a second 16x128 stationary matrix).

Device kernel (per core, batch-sharded 512 rows): one TensorE matmul per
(channel, m) chunk computes 128 subcarriers x 512 batch into PSUM; DVE and
ACT alternate evacuating psum to fp16 SBUF; chunked HWDGE stores stream the
8.4 MB fp16 output (half the f32 bytes -- the fp16 round-trip costs ~1e-3
relative error against a 2e-2 gate).

Schedule notes (v8, distilled from v1-v7 traces):
- ~17 us of runtime is fixed wrapper cost (engine barriers, TENSOR_LOADs,
  a mandatory ~254-instruction semaphore-reset epilogue): trivial kernel
  = 20.0 us.  Only the work phase is addressable.
- The store stream is SBUF-AXI-port-bound at ~427 GB/s (16 ports x
  27 GB/s); 8.4 MB takes ~19.6 us and dominates.  Loads into the
  17-partition band tiles touch only ~5 ports per band and pay a per-DMA
  sem-receipt barrier + non-pipelined HBM-read stalls, so the two bands
  load CONCURRENTLY: band 64 on sync (fastest ring), band 0 on gpsimd.
  The scalar ring is the slowest bulk loader (v3/v6: ~66-90 GB/s; a v6
  attempt to put band-0 there stalled the PE 5.3 us).
- Weights live in a DEDICATED tile: embedding them in the band tiles
  (v6/v7) made the Tile scheduler coarsen every LDWEIGHTS dependency
  onto the band's whole load stream (PE stalled until the last strip
  landed ~18 us).  But the weights DMA must not be one 81-row x 512 B
  monolith either -- its read-stalled descriptors gated the first matmul
  at ~11.4-12.2 us in v1-v5.  v8 loads the two 17-row halves as two
  small DMAs on the otherwise-idle scalar ring: w64 lands ~9.3 us.
- gpsimd (SWDGE) descriptor-ring traffic makes SDMA engine 15 lag the
  store tail; keeping all SWDGE work finished by ~16 us (fat pieces,
  strips split fine) keeps that tail ~1 us.
"""

import sys

import numpy as np

for _p in ("/opt/trn_rl_repo", "/root/.axon_site/_ro/trn_rl_repo"):
    if _p not in sys.path:
        sys.path.append(_p)

import concourse.bass as bass
import concourse.tile as tile
from concourse import bacc, mybir
from concourse.bass_utils import run_bass_kernel_spmd

N_CORES = 8
B, NP, NFFT, SPACING = 4096, 512, 4096, 8
B_LOC = B // N_CORES  # batch rows per core
P = 128  # SBUF partitions
NCHUNK = 64  # (ch, m) chunks: ch = q // 32 (re/im), m = q % 32 (128-subcarrier block)
MG = 16  # chunk slots per band
ROW = 2 * MG * B_LOC  # ls row length in fp16 elements (both channels)

# chunks per output store; small leading groups start the store stream
# early, 8-chunk middle groups give 8 KB-run descriptors (line rate),
# small trailing groups keep the post-compute drain short.
STORE_GROUPS = [2, 2, 4, 8, 8, 8, 8, 8, 8, 4, 2, 2]
assert sum(STORE_GROUPS) == NCHUNK

_PROGRAM = None


def _band_slot(m: int) -> tuple[int, int]:
    """SBUF band (partition base) and column slot of chunk m.

    Odd chunks (and m=31) live at partitions 0..16, even chunks at
    64..80 -- consecutive m alternate PE row-groups so back-to-back
    matmuls overlap in the array."""
    if m == 31:
        return 0, 15
    return (0, m // 2) if m % 2 else (64, m // 2)


def _build_program():
    """One Bass program, identical on all cores (pure data parallel)."""
    nc = bacc.Bacc("TRN2", target_bir_lowering=False, debug=False)
    f16 = mybir.dt.float16
    f32 = mybir.dt.float32
    # ls[17*band2 + j, ch*8192 + m2*512 + b]: band-major pre-gathered
    # layout (band2 0 = odd chunks -> partitions 0..16, band2 1 = even ->
    # 64..80).  Row = one partition's contiguous 32 KB payload.
    ls = nc.dram_tensor("ls", [34, ROW], f16, kind="ExternalInput").ap()
    # wm[81, 256]: cols 0:128 = W17 at rows 0..16 AND 64..80; cols
    # 128:256 rows 0..16 = the hN-folded last-chunk band.
    wm = nc.dram_tensor("wm", [81, 2 * P], f16, kind="ExternalInput").ap()
    # out[p, q*512 + b]: subcarrier-position p = 8*kk + r of chunk q = ch*32 + m.
    out = nc.dram_tensor("out", [P, NCHUNK * B_LOC], f16, kind="ExternalOutput").ap()

    with tile.TileContext(nc) as tc:
        with (
            tc.tile_pool(name="wpool", bufs=1) as wpool,
            tc.tile_pool(name="lpool", bufs=1) as lpool,
            tc.psum_pool(name="ppool", bufs=4) as ppool,
            tc.tile_pool(name="opool", bufs=6) as opool,
        ):
            # Dedicated weight tile -> LDWEIGHTS depends only on these two
            # small DMAs, never on the band data stream.
            wb = wpool.tile([81, 2 * P], f16, name="wb", tag="wb")
            nc.scalar.dma_start(wb[64:81, :], wm[64:81, :])
            nc.scalar.dma_start(wb[0:17, :], wm[0:17, :])

            # One tile per band holding both channels.  Mains are [16,
            # cols]; the 17th row (j=16) goes as [1, cols] strips (its
            # partition, 16 / 80, sits on its own SBUF AXI port, so
            # strips never contend with mains).  17-partition DMA
            # destinations would break the descriptor spray, hence 16+1.
            lts = {
                band: lpool.tile(
                    [81 if band else 17, ROW], f16,
                    name=f"ls{band}", tag=f"ls{band}",
                )
                for band in (64, 0)
            }

            def main(ring, band, c0, c1):
                lt = lts[band]
                r = 17 if band else 0
                ring.dma_start(lt[band : band + 16, c0:c1], ls[r : r + 16, c0:c1])

            def strip(ring, band, c0, c1):
                lt = lts[band]
                r = (17 if band else 0) + 16
                ring.dma_start(lt[band + 16 : band + 17, c0:c1], ls[r : r + 1, c0:c1])

            def col(ch, s):  # ls column of (channel, slot) batch block start
                return (ch * MG + s) * B_LOC

            # Pieces sized to the PE's slot-consumption deadlines: A =
            # ch0 slots 0-3 (gates the first matmuls), B1/B2 = ch0 rest,
            # C = ch1 (consumed from ~20 us).  Band 64 rides sync, band 0
            # rides gpsimd: the bands' disjoint SBUF-port groups (1,3,5,7
            # vs 0,2,4,6) drain concurrently.  Strips mirror the mains on
            # the same ring; their ports (8/9) are separate, so they
            # never contend with mains.  (Moving the ch1 pieces to the
            # scalar ring was measured 6 us SLOWER: scalar's slow
            # per-descriptor trickle poisons the ports during the store
            # stream -- loads must finish fast and early instead.)
            CUTS = [col(0, 0), col(0, 4), col(0, 10), col(1, 0), ROW]
            for band, ring in ((64, nc.sync), (0, nc.gpsimd)):
                for c0, c1 in zip(CUTS[:-1], CUTS[1:]):
                    if band == 64 and c0 == col(1, 0):
                        continue  # band-64 ch1 defers into the store stream
                    main(ring, band, c0, c1)
                    strip(ring, band, c0, c1)

            q = 0
            pair = 0
            for gi, gn in enumerate(STORE_GROUPS):
                o = opool.tile([P, gn * B_LOC], f16)
                q0 = q
                for j in range(0, gn, 2):
                    # one 2-bank psum tile per chunk pair: the paired evac
                    # (FD 1024) amortizes the per-op fixed cost and halves
                    # evac semaphore traffic.
                    ps = ppool.tile([P, 2 * B_LOC], f32)
                    for h in range(2):
                        ch, m = q // 32, q % 32
                        band, m2 = _band_slot(m)
                        if m == 31:
                            # last chunk: pilots 496..511, hN-folded band
                            lhsT, nrows = wb[0:16, P : 2 * P], 16
                        else:
                            lhsT, nrows = wb[band : band + 17, 0:P], 17
                        lt = lts[band]
                        c0 = col(ch, m2)
                        nc.tensor.matmul(
                            ps[:, h * B_LOC : (h + 1) * B_LOC],
                            lhsT,
                            lt[band : band + nrows, c0 : c0 + B_LOC],
                            start=True,
                            stop=True,
                        )
                        q += 1
                    # psum -> fp16 SBUF, alternating engines so consecutive
                    # pairs drain in parallel (DVE ~1.2us, ACT ~1.1us).
                    osl = o[:, j * B_LOC : (j + 2) * B_LOC]
                    if pair % 2 == 0:
                        nc.vector.tensor_copy(osl, ps[:])
                    else:
                        nc.scalar.copy(osl, ps[:])
                    pair += 1
                nc.sync.dma_start(out[:, q0 * B_LOC : q * B_LOC], o[:])
                if gi == 2:
                    # band-64 ch1 (deadline ~21.5 us, lands ~18) rides the
                    # sync FIFO BEHIND the first three store groups: the
                    # 13-17 us window then carries store bytes instead of
                    # a load piece nobody needs yet.
                    main(nc.sync, 64, col(1, 0), ROW)
                    strip(nc.sync, 64, col(1, 0), ROW)
    nc.compile()
    return nc


def _w_mats(decay_param) -> np.ndarray:
    """[81, 256] fp16: cols 0:128 = W17[j, 8kk+r] = alpha[r] (j=kk) /
    gamma[r] (j=kk+1) at rows 0..16 and duplicated at rows 64..80; cols
    128:256 rows 0..15 = the last-chunk band (kk=15 columns use the
    hN-folded coefficients on pilots 510/511)."""
    x = float(np.asarray(decay_param).reshape(-1)[0])
    d = float(np.logaddexp(0.0, x))  # softplus
    r = np.arange(SPACING, dtype=np.float64)
    eps = 1e-12
    wl = np.exp(-d * r)
    wr = np.exp(-d * (float(SPACING) - r))
    w = wl + wr + eps
    alpha, gamma = wl / w, wr / w
    # last 8 subcarriers: i = 4088 + r, x0 = 4088, x1 = 4095 (gap of 7);
    # y1 = hN = (15/8) H[511] - (7/8) H[510]
    wl2 = np.exp(-d * r)
    wr2 = np.exp(-d * (7.0 - r))
    w2 = wl2 + wr2 + eps
    c511 = (wl2 + 1.875 * wr2) / w2
    c510 = -0.875 * wr2 / w2
    W17 = np.zeros((17, P), np.float64)
    Wlast = np.zeros((16, P), np.float64)
    cols = np.arange(SPACING)
    for kk in range(16):
        W17[kk, 8 * kk + cols] = alpha
        W17[kk + 1, 8 * kk + cols] = gamma
    for kk in range(15):
        Wlast[kk, 8 * kk + cols] = alpha
        Wlast[kk + 1, 8 * kk + cols] = gamma
    Wlast[14, 120:128] = c510
    Wlast[15, 120:128] = c511
    W = np.zeros((81, 2 * P), np.float16)
    W[0:17, 0:P] = W17.astype(np.float16)
    W[64:81, 0:P] = W17.astype(np.float16)
    W[0:16, P : 2 * P] = Wlast.astype(np.float16)
    return W


def _gather_ls4(shard: np.ndarray) -> np.ndarray:
    """[34, 16384] fp16 band-major: row 17*band2 + j holds partition
    (band2 ? 0 : 64) + j's payload -- cols ch*8192 + slot*512 + b, one
    contiguous 32 KB DRAM run per partition (few fat load DMAs)."""
    lsT = shard.transpose(2, 1, 0).astype(np.float16).reshape(2 * NP, B_LOC)
    j = np.arange(17)[:, None]  # [17, 1]
    m_odd = np.array([2 * m2 + 1 for m2 in range(15)] + [31])  # band2=0 slots
    m_even = np.arange(0, 32, 2)  # band2=1 slots
    rows_odd = np.minimum(16 * m_odd[None, :] + j, NP - 1)  # clip m31 j=16
    rows_even = 16 * m_even[None, :] + j
    out = np.empty((34, ROW), np.float16)
    for ch in range(2):
        base = 512 * ch
        cols = slice(ch * MG * B_LOC, (ch + 1) * MG * B_LOC)
        out[0:17, cols] = lsT[base + rows_odd].reshape(17, -1)
        out[17:34, cols] = lsT[base + rows_even].reshape(17, -1)
    return out


def kernel(LS_ri, pilot_pos=None, decay_param=None, Nfft=None, **_unused):
    global _PROGRAM
    LS_ri = np.asarray(LS_ri, dtype=np.float32)
    Wm = _w_mats(decay_param)

    if _PROGRAM is None:
        _PROGRAM = _build_program()
    nc = _PROGRAM

    in_maps = []
    for c in range(N_CORES):
        shard = LS_ri[c * B_LOC : (c + 1) * B_LOC]  # [512, 512, 2]
        in_maps.append({"ls": _gather_ls4(shard), "wm": Wm})

    res = run_bass_kernel_spmd(nc, in_maps, list(range(N_CORES))).results
    outs = []
    for c in range(N_CORES):
        a = np.asarray(res[c]["out"]).reshape(16, 8, 2, 32, B_LOC)  # kk r ch m b
        a = a.transpose(4, 3, 0, 1, 2).reshape(B_LOC, NFFT, 2)
        outs.append(a.astype(np.float32))
    return np.concatenate(outs, axis=0)
